# Initial kernel scaffold
#
"""Channelwise tensor product (e3nn-style) GNN message passing on 8 TRN2 cores.

kernel(**inputs) takes the full (unsharded) problem and returns
(out0, out1) matching the reference:
    out0: (num_nodes, 64, 1) f32,  out1: (num_nodes, 64, 3) f32

Strategy (per sharding hint: partition edges, replicate node features):
 - Host: bucket edges by receiver (core = receiver // 6250, then 128-node
   output tile within core), split each bucket lo/hi by sender < 32768 so
   the int16-indexed dma_gather op can fetch x1[sender] rows, pad each
   group to 128-edge tiles with zero-weight edges, and prefold x2_0e and
   CG normalization into the per-edge weights (bf16).
 - Device (SPMD, identical program on 8 cores, no collectives; receiver
   ranges are disjoint): stream weights, dma_gather x1 rows (bf16),
   build the four tensor-product paths on DVE/ACT with k-major 1o layout,
   build one-hot receiver selection matrices via tensor_scalar is_equal
   against a constant iota, segment-sum via PE matmul accumulation into
   one PSUM tile per 128-node bucket, write each bucket tile once.
"""
import numpy as np
import ml_dtypes
from contextlib import ExitStack

import concourse.bass as bass
import concourse.bacc as bacc
import concourse.tile as tile
from concourse import mybir
from concourse.bass_utils import run_bass_kernel_spmd

BF16 = mybir.dt.bfloat16
F32 = mybir.dt.float32
I16 = mybir.dt.int16
NPBF = ml_dtypes.bfloat16

NCORES = 8
MUL = 32
VLO_CAP = 32768
INV_SQRT3 = np.float32(1.0 / np.sqrt(3.0, dtype=np.float32))


def _plan_and_shard(weights, x1_0e, x1_1o, x2_0e, x2_1o, senders, receivers,
                    num_nodes):
    E = weights.shape[0]
    N = int(num_nodes)
    NLOC = (N + NCORES - 1) // NCORES
    NTB = (NLOC + 127) // 128
    VLO = min(VLO_CAP, N)

    senders = np.asarray(senders).astype(np.int64)
    receivers = np.asarray(receivers).astype(np.int64)
    w = np.asarray(weights, dtype=np.float32).reshape(E, 4, MUL)
    sh0 = np.asarray(x2_0e, dtype=np.float32).reshape(E, 1)
    sh1 = np.asarray(x2_1o, dtype=np.float32).reshape(E, 3)

    core = np.minimum(receivers // NLOC, NCORES - 1)
    rloc = receivers - core * NLOC
    bucket = rloc >> 7
    hi = (senders >= VLO).astype(np.int64)

    key = (core * NTB + bucket) * 2 + hi
    ngroups = NCORES * NTB * 2
    counts = np.bincount(key, minlength=ngroups).reshape(NCORES, NTB, 2)

    T_lo = np.maximum((counts[:, :, 0].max(axis=0) + 127) // 128, 1)
    T_hi = np.maximum((counts[:, :, 1].max(axis=0) + 127) // 128, 1)
    T_b = T_lo + T_hi
    TT = int(T_b.sum())
    EP = TT * 128

    tile_base = np.concatenate([[0], np.cumsum(T_b)[:-1]])
    base_lo = tile_base * 128
    base_hi = base_lo + T_lo * 128

    order = np.argsort(key, kind='stable')
    sorted_key = key[order]
    grp_start = np.searchsorted(sorted_key, np.arange(ngroups), side='left')
    rank = np.empty(E, np.int64)
    rank[order] = np.arange(E) - grp_start[sorted_key]
    slot = np.where(hi == 0, base_lo[bucket], base_hi[bucket]) + rank

    wcat = np.empty((E, 128), dtype=np.float32)
    wcat[:, 0:32] = w[:, 0] * sh0
    wcat[:, 32:64] = w[:, 1]
    wcat[:, 64:96] = w[:, 2] * sh0
    wcat[:, 96:128] = w[:, 3] * INV_SQRT3

    x1cat = np.empty((N, 128), dtype=np.float32)
    x1cat[:, 0:32] = np.asarray(x1_0e, np.float32).reshape(N, MUL)
    x1cat[:, 32:128] = np.asarray(x1_1o, np.float32).reshape(N, MUL, 3) \
        .transpose(0, 2, 1).reshape(N, 96)
    x1cat = x1cat.astype(NPBF)

    iota = np.tile(np.arange(128, dtype=np.float32).astype(NPBF)[None, :],
                   (128, 1)).copy()

    in_maps = []
    sidx_local = np.where(hi == 0, senders, senders - VLO).astype(np.int16)
    recv_rel = (rloc - (bucket << 7)).astype(np.float32)
    for k in range(NCORES):
        m = core == k
        sl = slot[m]
        wpad = np.zeros((EP, 128), dtype=NPBF)
        wpad[sl] = wcat[m].astype(NPBF)
        sh1pad = np.zeros((EP, 3), dtype=NPBF)
        sh1pad[sl] = sh1[m].astype(NPBF)
        rrpad = np.zeros(EP, dtype=np.float32)
        rrpad[sl] = recv_rel[m]
        sipad = np.zeros(EP, dtype=np.int16)
        sipad[sl] = sidx_local[m]

        wdev = wpad.reshape(TT, 128, 128).transpose(1, 0, 2) \
            .reshape(128, TT * 128).copy()
        sh1dev = sh1pad.reshape(TT, 128, 3).transpose(1, 0, 2) \
            .reshape(128, TT * 3).copy()
        rrdev = rrpad.reshape(TT, 128).T.copy()

        idx = np.zeros((128, TT * 8), dtype=np.int16)
        off = 0
        for b in range(NTB):
            for Tg, b0 in ((int(T_lo[b]), int(base_lo[b])),
                           (int(T_hi[b]), int(base_hi[b]))):
                n = Tg * 128
                g = np.arange(n)
                idx[g % 16, off + g // 16] = sipad[b0:b0 + n]
                off += Tg * 8
        idx[16:32] = idx[:16]

        in_maps.append({
            "wdev": wdev, "sh1dev": sh1dev, "rrdev": rrdev, "idx": idx,
            "x1cat": x1cat, "iota": iota,
        })

    meta = dict(N=N, NLOC=NLOC, NTB=NTB, VLO=VLO,
                T_lo=[int(v) for v in T_lo], T_hi=[int(v) for v in T_hi],
                TT=TT)
    return meta, in_maps


def _build_program(meta):
    N, NTB, VLO, TT = meta["N"], meta["NTB"], meta["VLO"], meta["TT"]
    T_lo, T_hi = meta["T_lo"], meta["T_hi"]

    nc = bacc.Bacc("TRN2", target_bir_lowering=False, debug=False,
                   num_devices=NCORES)
    wdev_d = nc.dram_tensor("wdev", [128, TT * 128], BF16, kind="ExternalInput").ap()
    sh1_d = nc.dram_tensor("sh1dev", [128, TT * 3], BF16, kind="ExternalInput").ap()
    rr_d = nc.dram_tensor("rrdev", [128, TT], F32, kind="ExternalInput").ap()
    idx_d = nc.dram_tensor("idx", [128, TT * 8], I16, kind="ExternalInput").ap()
    x1_d = nc.dram_tensor("x1cat", [N, 128], BF16, kind="ExternalInput").ap()
    iota_d = nc.dram_tensor("iota", [128, 128], BF16, kind="ExternalInput").ap()
    out_d = nc.dram_tensor("out", [NTB, 128, 256], F32, kind="ExternalOutput").ap()

    mm = mybir.AluOpType.mult
    with tile.TileContext(nc) as tc:
        with ExitStack() as ctx:
            cpool = ctx.enter_context(tc.tile_pool(name="const", bufs=1))
            gpool = ctx.enter_context(tc.tile_pool(name="gath", bufs=3))
            wpool = ctx.enter_context(tc.tile_pool(name="wts", bufs=3))
            epool = ctx.enter_context(tc.tile_pool(name="exp", bufs=2))
            tpool = ctx.enter_context(tc.tile_pool(name="tmp", bufs=2))
            mpool = ctx.enter_context(tc.tile_pool(name="msg", bufs=3))
            spool = ctx.enter_context(tc.tile_pool(name="sel", bufs=6))
            ppool = ctx.enter_context(tc.tile_pool(name="psum", bufs=4, space="PSUM"))
            opool = ctx.enter_context(tc.tile_pool(name="outs", bufs=3))

            idx_sb = cpool.tile([128, TT * 8], I16)
            nc.sync.dma_start(idx_sb[:], idx_d[:])
            rr_sb = cpool.tile([128, TT], F32)
            nc.sync.dma_start(rr_sb[:], rr_d[:])
            sh1_sb = cpool.tile([128, TT * 3], BF16)
            nc.sync.dma_start(sh1_sb[:], sh1_d[:])
            iota_sb = cpool.tile([128, 128], BF16)
            nc.sync.dma_start(iota_sb[:], iota_d[:])

            t0 = 0
            for b in range(NTB):
                Tl, Th = T_lo[b], T_hi[b]
                T = Tl + Th
                g = gpool.tile([128, T, 128], BF16, tag="g")
                nc.gpsimd.dma_gather(
                    g[:, 0:Tl, :], x1_d[0:VLO, :],
                    idx_sb[:, t0 * 8:(t0 + Tl) * 8],
                    num_idxs=Tl * 128, num_idxs_reg=Tl * 128, elem_size=128,
                    single_packet=False)
                nc.gpsimd.dma_gather(
                    g[:, Tl:T, :], x1_d[VLO:N, :],
                    idx_sb[:, (t0 + Tl) * 8:(t0 + T) * 8],
                    num_idxs=Th * 128, num_idxs_reg=Th * 128, elem_size=128,
                    single_packet=False)
                w = wpool.tile([128, T, 128], BF16, tag="w")
                nc.sync.dma_start(
                    w[:], wdev_d[:, t0 * 128:(t0 + T) * 128]
                    .rearrange("p (t f) -> p t f", f=128))

                s0 = g[:, :, 0:32]
                s1 = g[:, :, 32:128].rearrange("p t (k c) -> p t k c", c=32)
                A0 = w[:, :, 0:32]
                w1 = w[:, :, 32:64]
                A2 = w[:, :, 64:96]
                w3 = w[:, :, 96:128]

                she = epool.tile([128, T, 3, 32], BF16, tag="she")
                nc.scalar.copy(
                    she[:],
                    sh1_sb[:, t0 * 3:(t0 + T) * 3]
                    .rearrange("p (t k) -> p t k", k=3)
                    .unsqueeze(3).to_broadcast([128, T, 3, 32]))

                msg = mpool.tile([128, T, 256], BF16, tag="msg")
                nc.vector.tensor_tensor(out=msg[:, :, 0:32], in0=A0, in1=s0, op=mm)
                t3 = tpool.tile([128, T, 3, 32], BF16, tag="t3")
                nc.vector.tensor_tensor(out=t3[:], in0=s1, in1=she[:], op=mm)
                dot = tpool.tile([128, T, 32], BF16, tag="dot")
                nc.vector.tensor_add(out=dot[:], in0=t3[:, :, 0, :], in1=t3[:, :, 1, :])
                nc.vector.tensor_add(out=dot[:], in0=dot[:], in1=t3[:, :, 2, :])
                nc.vector.tensor_tensor(out=msg[:, :, 32:64], in0=dot[:], in1=w3, op=mm)
                t1 = tpool.tile([128, T, 32], BF16, tag="t1")
                nc.vector.tensor_tensor(out=t1[:], in0=w1, in1=s0, op=mm)
                nc.vector.tensor_tensor(
                    out=msg[:, :, 64:160].rearrange("p t (k c) -> p t k c", c=32),
                    in0=t1[:].unsqueeze(2).to_broadcast([128, T, 3, 32]),
                    in1=she[:], op=mm)
                nc.vector.tensor_tensor(
                    out=msg[:, :, 160:256].rearrange("p t (k c) -> p t k c", c=32),
                    in0=A2.unsqueeze(2).to_broadcast([128, T, 3, 32]),
                    in1=s1, op=mm)

                ps = ppool.tile([128, 256], F32, tag="ps")
                for t in range(T):
                    S = spool.tile([128, 128], BF16, tag="S")
                    nc.vector.tensor_scalar(
                        out=S[:], in0=iota_sb[:],
                        scalar1=rr_sb[:, t0 + t:t0 + t + 1], scalar2=None,
                        op0=mybir.AluOpType.is_equal)
                    nc.tensor.matmul(ps[:], lhsT=S[:], rhs=msg[:, t, :],
                                     start=(t == 0), stop=(t == T - 1))
                ob = opool.tile([128, 256], F32, tag="ob")
                nc.scalar.copy(ob[:], ps[:])
                nc.sync.dma_start(out_d[b], ob[:])
                t0 += T
    nc.compile()
    return nc


def _postprocess(meta, results):
    N, NLOC, NTB = meta["N"], meta["NLOC"], meta["NTB"]
    outs = []
    for k in range(NCORES):
        o = results[k]["out"].reshape(NTB * 128, 256)
        lo = k * NLOC
        outs.append(o[:min(NLOC, N - lo)])
    o = np.concatenate(outs, axis=0)
    out0 = np.ascontiguousarray(o[:, 0:64]).reshape(N, 64, 1).astype(np.float32)
    m1a = o[:, 64:160].reshape(N, 3, 32).transpose(0, 2, 1)
    m1b = o[:, 160:256].reshape(N, 3, 32).transpose(0, 2, 1)
    out1 = np.ascontiguousarray(np.concatenate([m1a, m1b], axis=1)).astype(np.float32)
    return out0, out1


def kernel(weights, x1_0e, x1_1o, x2_0e, x2_1o, senders, receivers, num_nodes,
           trace=False, tmpdir=None):
    meta, in_maps = _plan_and_shard(weights, x1_0e, x1_1o, x2_0e, x2_1o,
                                    senders, receivers, num_nodes)
    nc = _build_program(meta)
    res = run_bass_kernel_spmd(nc, in_maps, list(range(NCORES)),
                               trace=trace, tmpdir=tmpdir)
    out = _postprocess(meta, res.results)
    if trace:
        return out, res
    return out


# revision 1
# speedup vs baseline: 1.0614x; 1.0614x over previous
"""Channelwise tensor product (e3nn-style) GNN message passing on 8 TRN2 cores.

kernel(**inputs) takes the full (unsharded) problem and returns
(out0, out1) matching the reference:
    out0: (num_nodes, 64, 1) f32,  out1: (num_nodes, 64, 3) f32

Strategy (per sharding hint: partition edges, replicate node features):
 - Host: bucket edges by receiver (core = receiver // 6250, then 128-node
   output tile within core), split each bucket lo/hi by sender < 32768 so
   the int16-indexed dma_gather op can fetch x1[sender] rows, pad each
   group to 128-edge tiles with zero-weight edges, and prefold x2_0e and
   CG normalization into the per-edge weights (bf16).
 - Device (SPMD, identical program on 8 cores, no collectives; receiver
   ranges are disjoint): stream weights, dma_gather x1 rows (bf16),
   build the four tensor-product paths on DVE/ACT with k-major 1o layout,
   build one-hot receiver selection matrices via tensor_scalar is_equal
   against a constant iota, segment-sum via PE matmul accumulation into
   one PSUM tile per 128-node bucket, write each bucket tile once.
"""
import numpy as np
import ml_dtypes
from contextlib import ExitStack

import concourse.bass as bass
import concourse.bacc as bacc
import concourse.tile as tile
from concourse import mybir
from concourse.bass_utils import run_bass_kernel_spmd

BF16 = mybir.dt.bfloat16
F32 = mybir.dt.float32
I16 = mybir.dt.int16
NPBF = ml_dtypes.bfloat16

NCORES = 8
MUL = 32
VLO_CAP = 32768
INV_SQRT3 = np.float32(1.0 / np.sqrt(3.0, dtype=np.float32))


def _plan_and_shard(weights, x1_0e, x1_1o, x2_0e, x2_1o, senders, receivers,
                    num_nodes):
    E = weights.shape[0]
    N = int(num_nodes)
    NLOC = (N + NCORES - 1) // NCORES
    NTB = (NLOC + 127) // 128
    VLO = min(VLO_CAP, N)

    senders = np.asarray(senders).astype(np.int64)
    receivers = np.asarray(receivers).astype(np.int64)
    w = np.asarray(weights, dtype=np.float32).reshape(E, 4, MUL)
    sh0 = np.asarray(x2_0e, dtype=np.float32).reshape(E, 1)
    sh1 = np.asarray(x2_1o, dtype=np.float32).reshape(E, 3)

    core = np.minimum(receivers // NLOC, NCORES - 1)
    rloc = receivers - core * NLOC
    bucket = rloc >> 7
    hi = (senders >= VLO).astype(np.int64)

    key = (core * NTB + bucket) * 2 + hi
    ngroups = NCORES * NTB * 2
    counts = np.bincount(key, minlength=ngroups).reshape(NCORES, NTB, 2)

    T_lo = np.maximum((counts[:, :, 0].max(axis=0) + 127) // 128, 1)
    T_hi = np.maximum((counts[:, :, 1].max(axis=0) + 127) // 128, 1)
    T_b = T_lo + T_hi
    TT = int(T_b.sum())
    EP = TT * 128

    tile_base = np.concatenate([[0], np.cumsum(T_b)[:-1]])
    base_lo = tile_base * 128
    base_hi = base_lo + T_lo * 128

    order = np.argsort(key, kind='stable')
    sorted_key = key[order]
    grp_start = np.searchsorted(sorted_key, np.arange(ngroups), side='left')
    rank = np.empty(E, np.int64)
    rank[order] = np.arange(E) - grp_start[sorted_key]
    slot = np.where(hi == 0, base_lo[bucket], base_hi[bucket]) + rank

    wcat = np.empty((E, 128), dtype=np.float32)
    wcat[:, 0:32] = w[:, 0] * sh0
    wcat[:, 32:64] = w[:, 1]
    wcat[:, 64:96] = w[:, 2] * sh0
    wcat[:, 96:128] = w[:, 3] * INV_SQRT3

    x1cat = np.empty((N, 128), dtype=np.float32)
    x1cat[:, 0:32] = np.asarray(x1_0e, np.float32).reshape(N, MUL)
    x1cat[:, 32:128] = np.asarray(x1_1o, np.float32).reshape(N, MUL, 3) \
        .transpose(0, 2, 1).reshape(N, 96)
    x1cat = x1cat.astype(NPBF)

    iota = np.tile(np.arange(128, dtype=np.float32).astype(NPBF)[None, :],
                   (128, 1)).copy()

    in_maps = []
    sidx_local = np.where(hi == 0, senders, senders - VLO).astype(np.int16)
    recv_rel = (rloc - (bucket << 7)).astype(np.float32)
    for k in range(NCORES):
        m = core == k
        sl = slot[m]
        wpad = np.zeros((EP, 128), dtype=NPBF)
        wpad[sl] = wcat[m].astype(NPBF)
        sh1pad = np.zeros((EP, 3), dtype=NPBF)
        sh1pad[sl] = sh1[m].astype(NPBF)
        rrpad = np.zeros(EP, dtype=np.float32)
        rrpad[sl] = recv_rel[m]
        sipad = np.zeros(EP, dtype=np.int16)
        sipad[sl] = sidx_local[m]

        wdev = wpad.reshape(TT, 128, 128).transpose(1, 0, 2) \
            .reshape(128, TT * 128).copy()
        sh1dev = sh1pad.reshape(TT, 128, 3).transpose(1, 0, 2) \
            .reshape(128, TT * 3).copy()
        rrdev = rrpad.reshape(TT, 128).T.copy()

        idx = np.zeros((128, TT * 8), dtype=np.int16)
        off = 0
        for b in range(NTB):
            for Tg, b0 in ((int(T_lo[b]), int(base_lo[b])),
                           (int(T_hi[b]), int(base_hi[b]))):
                n = Tg * 128
                g = np.arange(n)
                idx[g % 16, off + g // 16] = sipad[b0:b0 + n]
                off += Tg * 8
        idx[16:32] = idx[:16]

        in_maps.append({
            "wdev": wdev, "sh1dev": sh1dev, "rrdev": rrdev, "idx": idx,
            "x1cat": x1cat, "iota": iota,
        })

    meta = dict(N=N, NLOC=NLOC, NTB=NTB, VLO=VLO,
                T_lo=[int(v) for v in T_lo], T_hi=[int(v) for v in T_hi],
                TT=TT)
    return meta, in_maps


def _build_program(meta):
    N, NTB, VLO, TT = meta["N"], meta["NTB"], meta["VLO"], meta["TT"]
    T_lo, T_hi = meta["T_lo"], meta["T_hi"]

    nc = bacc.Bacc("TRN2", target_bir_lowering=False, debug=False,
                   num_devices=NCORES)
    wdev_d = nc.dram_tensor("wdev", [128, TT * 128], BF16, kind="ExternalInput").ap()
    sh1_d = nc.dram_tensor("sh1dev", [128, TT * 3], BF16, kind="ExternalInput").ap()
    rr_d = nc.dram_tensor("rrdev", [128, TT], F32, kind="ExternalInput").ap()
    idx_d = nc.dram_tensor("idx", [128, TT * 8], I16, kind="ExternalInput").ap()
    x1_d = nc.dram_tensor("x1cat", [N, 128], BF16, kind="ExternalInput").ap()
    iota_d = nc.dram_tensor("iota", [128, 128], BF16, kind="ExternalInput").ap()
    out_d = nc.dram_tensor("out", [NTB, 128, 256], F32, kind="ExternalOutput").ap()

    mm = mybir.AluOpType.mult
    with tile.TileContext(nc) as tc:
        with ExitStack() as ctx:
            cpool = ctx.enter_context(tc.tile_pool(name="const", bufs=1))
            gpool = ctx.enter_context(tc.tile_pool(name="gath", bufs=3))
            wpool = ctx.enter_context(tc.tile_pool(name="wts", bufs=3))
            epool = ctx.enter_context(tc.tile_pool(name="exp", bufs=2))
            tpool = ctx.enter_context(tc.tile_pool(name="tmp", bufs=2))
            mpool = ctx.enter_context(tc.tile_pool(name="msg", bufs=3))
            spool = ctx.enter_context(tc.tile_pool(name="sel", bufs=6))
            ppool = ctx.enter_context(tc.tile_pool(name="psum", bufs=4, space="PSUM"))
            opool = ctx.enter_context(tc.tile_pool(name="outs", bufs=3))

            idx_sb = cpool.tile([128, TT * 8], I16)
            nc.sync.dma_start(idx_sb[:], idx_d[:])
            rr_sb = cpool.tile([128, TT], F32)
            nc.sync.dma_start(rr_sb[:], rr_d[:])
            sh1_sb = cpool.tile([128, TT * 3], BF16)
            nc.sync.dma_start(sh1_sb[:], sh1_d[:])
            iota_sb = cpool.tile([128, 128], BF16)
            nc.sync.dma_start(iota_sb[:], iota_d[:])

            t0 = 0
            for b in range(NTB):
                Tl, Th = T_lo[b], T_hi[b]
                T = Tl + Th
                g = gpool.tile([128, T, 128], BF16, tag="g")
                nc.gpsimd.dma_gather(
                    g[:, 0:Tl, :], x1_d[0:VLO, :],
                    idx_sb[:, t0 * 8:(t0 + Tl) * 8],
                    num_idxs=Tl * 128, num_idxs_reg=Tl * 128, elem_size=128,
                    single_packet=False)
                nc.gpsimd.dma_gather(
                    g[:, Tl:T, :], x1_d[VLO:N, :],
                    idx_sb[:, (t0 + Tl) * 8:(t0 + T) * 8],
                    num_idxs=Th * 128, num_idxs_reg=Th * 128, elem_size=128,
                    single_packet=False)
                w = wpool.tile([128, T, 128], BF16, tag="w")
                nc.sync.dma_start(
                    w[:], wdev_d[:, t0 * 128:(t0 + T) * 128]
                    .rearrange("p (t f) -> p t f", f=128))

                s0 = g[:, :, 0:32]
                s1 = g[:, :, 32:128].rearrange("p t (k c) -> p t k c", c=32)
                A0 = w[:, :, 0:32]
                w1 = w[:, :, 32:64]
                A2 = w[:, :, 64:96]
                w3 = w[:, :, 96:128]

                she = epool.tile([128, T, 3, 32], BF16, tag="she")
                nc.scalar.copy(
                    she[:],
                    sh1_sb[:, t0 * 3:(t0 + T) * 3]
                    .rearrange("p (t k) -> p t k", k=3)
                    .unsqueeze(3).to_broadcast([128, T, 3, 32]))

                msg = mpool.tile([128, T, 256], BF16, tag="msg")
                nc.vector.tensor_tensor(out=msg[:, :, 0:32], in0=A0, in1=s0, op=mm)
                t3 = tpool.tile([128, T, 3, 32], BF16, tag="t3")
                nc.vector.tensor_tensor(out=t3[:], in0=s1, in1=she[:], op=mm)
                dot = tpool.tile([128, T, 32], BF16, tag="dot")
                nc.vector.tensor_add(out=dot[:], in0=t3[:, :, 0, :], in1=t3[:, :, 1, :])
                nc.vector.tensor_add(out=dot[:], in0=dot[:], in1=t3[:, :, 2, :])
                nc.vector.tensor_tensor(out=msg[:, :, 32:64], in0=dot[:], in1=w3, op=mm)
                t1 = tpool.tile([128, T, 32], BF16, tag="t1")
                nc.vector.tensor_tensor(out=t1[:], in0=w1, in1=s0, op=mm)
                nc.vector.tensor_tensor(
                    out=msg[:, :, 64:160].rearrange("p t (k c) -> p t k c", c=32),
                    in0=t1[:].unsqueeze(2).to_broadcast([128, T, 3, 32]),
                    in1=she[:], op=mm)
                nc.vector.tensor_tensor(
                    out=msg[:, :, 160:256].rearrange("p t (k c) -> p t k c", c=32),
                    in0=A2.unsqueeze(2).to_broadcast([128, T, 3, 32]),
                    in1=s1, op=mm)

                ps = ppool.tile([128, 256], F32, tag="ps")
                for t in range(T):
                    S = spool.tile([128, 128], BF16, tag="S")
                    nc.vector.tensor_scalar(
                        out=S[:], in0=iota_sb[:],
                        scalar1=rr_sb[:, t0 + t:t0 + t + 1], scalar2=None,
                        op0=mybir.AluOpType.is_equal)
                    nc.tensor.matmul(ps[:], lhsT=S[:], rhs=msg[:, t, :],
                                     start=(t == 0), stop=(t == T - 1))
                ob = opool.tile([128, 256], F32, tag="ob")
                nc.scalar.copy(ob[:], ps[:])
                nc.sync.dma_start(out_d[b], ob[:])
                t0 += T
    nc.compile()
    return nc


def _postprocess(meta, results):
    N, NLOC, NTB = meta["N"], meta["NLOC"], meta["NTB"]
    outs = []
    for k in range(NCORES):
        o = results[k]["out"].reshape(NTB * 128, 256)
        lo = k * NLOC
        outs.append(o[:min(NLOC, N - lo)])
    o = np.concatenate(outs, axis=0)
    out0 = np.ascontiguousarray(o[:, 0:64]).reshape(N, 64, 1).astype(np.float32)
    m1a = o[:, 64:160].reshape(N, 3, 32).transpose(0, 2, 1)
    m1b = o[:, 160:256].reshape(N, 3, 32).transpose(0, 2, 1)
    out1 = np.ascontiguousarray(np.concatenate([m1a, m1b], axis=1)).astype(np.float32)
    return out0, out1


def kernel(weights, x1_0e, x1_1o, x2_0e, x2_1o, senders, receivers, num_nodes,
           trace=False, tmpdir=None):
    meta, in_maps = _plan_and_shard(weights, x1_0e, x1_1o, x2_0e, x2_1o,
                                    senders, receivers, num_nodes)
    nc = _build_program(meta)
    res = run_bass_kernel_spmd(nc, in_maps, list(range(NCORES)),
                               trace=trace, tmpdir=tmpdir)
    out = _postprocess(meta, res.results)
    if trace:
        return out, res
    return out
